# revision 10
# baseline (speedup 1.0000x reference)
"""GraphSAGE (2-layer, DGL SAGEConv-mean) Trainium2 kernel — y-scheme.

Data-parallel over B (4 samples per core, 8 cores). Per (b,c) pair, with
A=adj, deg=max(indeg,1), D=diag(deg):

  y  = A^T x                      (level Y, 24 cols/pair)
  [R1'|R4|R5] = y @ [A00|B01|C01] (PE transpose of y + small matmuls,
                                   output lands node-major directly)
  t  = A^T (D^{-1} R5)            (level T2)
  w  = A^T R5                     (level W)
  OUT0 = dinv4*(t + R4) + (4*x@A00 + biasN)        [host-folded mb0]
  OUT1 = dinv*(A^T (R4 + D^{-1} w)) + R1' + biasN  (level A1)

vs the previous 6-level scheme this applies A^T to 4 slabs per pair
instead of 6 (96 vs 144 moving cols/pair). adj is stored fp8_e4m3
(exact for 0/1), halving its SBUF/DMA footprint. Small-weight products
use lhsT = y^T chunks so results come out node-major (no back-transpose).
"""
import sys

sys.path.insert(0, "/opt/trn_rl_repo")

import numpy as np
import ml_dtypes

from concourse import bass, bacc, tile, mybir
from concourse.bass_utils import run_bass_kernel_spmd

BF16 = mybir.dt.bfloat16
F32 = mybir.dt.float32
FP8 = mybir.dt.float8e4

N = 2048
L = 24
B = 32
C = 8
NCORES = 8
BSH = B // NCORES          # 4 samples per core
NPAIR = BSH * C            # 32 (b,c) pairs per core
NT = N // 128              # 16 node tiles
NG = 2                     # pair groups per core
GP = NPAIR // NG           # 16 pairs per group
GC = GP * L                # 384 moving columns per group
NSLAB = 4                  # transpose slabs per group (4 pairs each)
SP = GP // NSLAB           # pairs per slab
SW = SP * L                # 96 columns per slab

_CACHE = {}


def _build_bass():
    nc = bacc.Bacc(
        "TRN2", target_bir_lowering=False, debug=False, num_devices=NCORES)
    adjb = nc.declare_dram_parameter("adjb", [128, NT * N], FP8, isOutput=False)
    xsd = nc.declare_dram_parameter("xs", [NG, 128, NT * GC], BF16, isOutput=False)
    mbd = nc.declare_dram_parameter("mb0", [NG, 128, NT * GC], BF16, isOutput=False)
    dinvd = nc.declare_dram_parameter("dinv", [128, NT], F32, isOutput=False)
    dinv4d = nc.declare_dram_parameter("dinv4", [128, NT], F32, isOutput=False)
    biasd = nc.declare_dram_parameter("biasN", [128, NT * GC], BF16, isOutput=False)
    wpd = nc.declare_dram_parameter("wp", [128, 72], BF16, isOutput=False)
    idd = nc.declare_dram_parameter("ident", [128, 128], BF16, isOutput=False)
    od = nc.declare_dram_parameter("o", [NG, NT, 2, 128, GC], F32, isOutput=True)

    mult = mybir.AluOpType.mult
    add = mybir.AluOpType.add

    with tile.TileContext(nc) as tc:
        with (
            tc.tile_pool(name="cst", bufs=1) as cst,
            tc.tile_pool(name="adjp", bufs=1) as adjp,
            tc.tile_pool(name="mov", bufs=2) as mov,
            tc.tile_pool(name="ysp", bufs=1) as ysp,
            tc.tile_pool(name="ytp", bufs=1) as ytp,
            tc.tile_pool(name="rap", bufs=1) as rap,
            tc.tile_pool(name="wrk", bufs=1) as wrk,
            tc.tile_pool(name="otp", bufs=4) as otp,
            tc.tile_pool(name="psY", bufs=2, space="PSUM") as psY,
            tc.tile_pool(name="psT", bufs=2, space="PSUM") as psT,
            tc.tile_pool(name="psS", bufs=2, space="PSUM") as psS,
            tc.tile_pool(name="psB", bufs=2, space="PSUM") as psB,
        ):
            adj_sb = adjp.tile([128, NT * N], FP8)
            nc.sync.dma_start(adj_sb[:], adjb[:])
            dinv_sb = cst.tile([128, NT], F32, tag="dinv")
            nc.sync.dma_start(dinv_sb[:], dinvd[:])
            dinv4_sb = cst.tile([128, NT], F32, tag="dinv4")
            nc.sync.dma_start(dinv4_sb[:], dinv4d[:])
            bias_sb = cst.tile([128, NT * GC], BF16, tag="biasN")
            nc.sync.dma_start(bias_sb[:], biasd[:])
            wp_sb = cst.tile([128, 72], BF16, tag="wp")
            nc.sync.dma_start(wp_sb[:], wpd[:])
            id_sb = cst.tile([128, 128], BF16, tag="ident")
            nc.sync.dma_start(id_sb[:], idd[:])

            def astile(u, vt):
                col = u * N + vt * 128
                return adj_sb[:, col:col + 128]

            for g in range(NG):
                xg = mov.tile([128, NT * GC], BF16, tag="xg")
                mb = mov.tile([128, NT * GC], BF16, tag="mb")
                nc.sync.dma_start(xg[:], xsd[g])
                nc.sync.dma_start(mb[:], mbd[g])

                # Level Y: y = A^T x (node-major)
                ys = ysp.tile([128, NT * GC], BF16, tag="ys")
                for vt in range(NT):
                    ps = psY.tile([128, GC], F32)
                    for u in range(NT):
                        nc.tensor.matmul(
                            ps[:], astile(u, vt), xg[:, u * GC:(u + 1) * GC],
                            start=(u == 0), stop=(u == NT - 1))
                    nc.vector.tensor_copy(ys[:, vt * GC:(vt + 1) * GC], ps[:])

                # Transpose y -> y^T slabs (4 pairs per slab)
                yts = [
                    ytp.tile([128, NT * 128], BF16, tag=f"yt{s}",
                             name=f"yt{s}")
                    for s in range(NSLAB)
                ]
                # pair sp within a slab sits at partition base 32*sp (PE
                # tile positions must be 32-aligned)
                for ut in range(NT):
                    for s in range(NSLAB):
                        pt = psT.tile([128, 128], BF16)
                        for sp in range(SP):
                            p = s * SP + sp
                            nc.tensor.transpose(
                                pt[32 * sp:32 * sp + L, :],
                                ys[:, ut * GC + p * L: ut * GC + (p + 1) * L],
                                id_sb[:], tile_position=(0, 32 * sp))
                            nc.vector.tensor_copy(
                                yts[s][32 * sp:32 * sp + L,
                                       ut * 128:(ut + 1) * 128],
                                pt[32 * sp:32 * sp + L, :])

                # Smalls: [R1'|R4|R5](tile ut) = (y^T chunk)^T @ wp, node-major out
                rall = rap.tile([128, NT, GP, 72], BF16, tag="rall")
                for p in range(GP):
                    s, sp = divmod(p, SP)
                    for ut in range(NT):
                        pm = psS.tile([128, 72], F32)
                        nc.tensor.matmul(
                            pm[:],
                            yts[s][32 * sp:32 * sp + L,
                                   ut * 128:(ut + 1) * 128],
                            wp_sb[32 * sp:32 * sp + L, :],
                            tile_position=(32 * sp, 0))
                        nc.vector.tensor_copy(rall[:, ut, p, :], pm[:])

                # D5 = dinv * R5 (per source node)
                d5 = wrk.tile([128, NT * GC], BF16, tag="d5")
                for ut in range(NT):
                    nc.vector.tensor_scalar_mul(
                        d5[:, ut * GC:(ut + 1) * GC],
                        rall[:, ut, :, 48:72], dinv_sb[:, ut:ut + 1])

                # Level W: w = A^T R5; U2s = R4 + dinv*w
                u2s = wrk.tile([128, NT * GC], BF16, tag="u2s")
                for vt in range(NT):
                    ps = psB.tile([128, GC], F32)
                    for u in range(NT):
                        nc.tensor.matmul(
                            ps[:], astile(u, vt), rall[:, u, :, 48:72],
                            start=(u == 0), stop=(u == NT - 1))
                    nc.vector.scalar_tensor_tensor(
                        u2s[:, vt * GC:(vt + 1) * GC], ps[:],
                        dinv_sb[:, vt:vt + 1], rall[:, vt, :, 24:48],
                        op0=mult, op1=add)

                # Level T2: t = A^T D5; OUT0 = dinv4*(t + R4) + mb0
                for vt in range(NT):
                    ps = psB.tile([128, GC], F32)
                    for u in range(NT):
                        nc.tensor.matmul(
                            ps[:], astile(u, vt), d5[:, u * GC:(u + 1) * GC],
                            start=(u == 0), stop=(u == NT - 1))
                    t0 = otp.tile([128, GC], F32, tag="t0")
                    nc.vector.tensor_tensor(
                        t0[:], ps[:], rall[:, vt, :, 24:48], op=add)
                    t0b = otp.tile([128, GC], F32, tag="t0b")
                    nc.vector.scalar_tensor_tensor(
                        t0b[:], t0[:], dinv4_sb[:, vt:vt + 1],
                        mb[:, vt * GC:(vt + 1) * GC], op0=mult, op1=add)
                    nc.sync.dma_start(od[g, vt, 0], t0b[:])

                # Level A1: a1 = A^T U2s; OUT1 = dinv*a1 + R1' + biasN
                for vt in range(NT):
                    ps = psB.tile([128, GC], F32)
                    for u in range(NT):
                        nc.tensor.matmul(
                            ps[:], astile(u, vt), u2s[:, u * GC:(u + 1) * GC],
                            start=(u == 0), stop=(u == NT - 1))
                    t1 = otp.tile([128, GC], F32, tag="t1")
                    nc.vector.scalar_tensor_tensor(
                        t1[:], ps[:], dinv_sb[:, vt:vt + 1],
                        rall[:, vt, :, 0:24], op0=mult, op1=add)
                    t1b = otp.tile([128, GC], F32, tag="t1b")
                    nc.vector.tensor_tensor(
                        t1b[:], t1[:], bias_sb[:, vt * GC:(vt + 1) * GC], op=add)
                    nc.sync.dma_start(od[g, vt, 1], t1b[:])
    nc.compile()
    return nc


def _pack_moving(m):
    """[BSH, C, N, L] f32 -> [NG, 128, NT*GC] bf16 (pairs b-major)."""
    a = m.transpose(2, 0, 1, 3).reshape(NT, 128, NPAIR * L)
    a = a.reshape(NT, 128, NG, GC).transpose(2, 1, 0, 3).reshape(NG, 128, NT * GC)
    return np.ascontiguousarray(a).astype(ml_dtypes.bfloat16)


def kernel(x, adj, W_self, W_neigh, bias, _trace=False):
    x = np.asarray(x, dtype=np.float32)
    adj = np.asarray(adj, dtype=np.float32)
    W_self = np.asarray(W_self, dtype=np.float32)
    W_neigh = np.asarray(W_neigh, dtype=np.float32)
    bias = np.asarray(bias, dtype=np.float32)

    A00 = W_self[0].T @ W_self[1].T
    B01 = W_neigh[0].T @ W_self[1].T + W_self[0].T @ W_neigh[1].T
    C01 = W_neigh[0].T @ W_neigh[1].T
    indeg = adj.sum(0)
    deg = np.maximum(indeg, 1.0)
    s = (indeg >= 1).astype(np.float32)
    biasN = (bias[0] @ W_self[1].T + bias[1])[None, :] \
        + s[:, None] * (bias[0] @ W_neigh[1].T)[None, :]      # [N, L]

    adjb = np.ascontiguousarray(
        adj.reshape(NT, 128, N).transpose(1, 0, 2).reshape(128, NT * N)
    ).astype(ml_dtypes.float8_e4m3)
    dinv = np.ascontiguousarray((1.0 / deg).reshape(NT, 128).T).astype(np.float32)
    dinv4 = np.ascontiguousarray(4.0 * dinv)
    biasP = np.ascontiguousarray(
        np.broadcast_to(biasN.reshape(NT, 128, 1, L), (NT, 128, GP, L))
        .reshape(NT, 128, GC).transpose(1, 0, 2).reshape(128, NT * GC)
    ).astype(ml_dtypes.bfloat16)
    wp1 = np.concatenate([A00, B01, C01], axis=1)        # [24, 72]
    wp = np.zeros((128, 72), dtype=np.float32)
    for k in range(4):
        wp[32 * k:32 * k + L] = wp1
    wp = wp.astype(ml_dtypes.bfloat16)
    ident = np.eye(128, dtype=np.float32).astype(ml_dtypes.bfloat16)

    mb_all = 4.0 * (x @ A00) + biasN[None, None]

    if "nc" not in _CACHE:
        _CACHE["nc"] = _build_bass()
    nc = _CACHE["nc"]

    in_maps = []
    for c in range(NCORES):
        sl = slice(c * BSH, (c + 1) * BSH)
        in_maps.append({
            "adjb": adjb,
            "xs": _pack_moving(x[sl]),
            "mb0": _pack_moving(mb_all[sl]),
            "dinv": dinv,
            "dinv4": dinv4,
            "biasN": biasP,
            "wp": wp,
            "ident": ident,
        })

    res = run_bass_kernel_spmd(
        nc, in_maps, list(range(NCORES)), trace=_trace)

    out = np.empty((B, 2 * C, N, L), dtype=np.float32)
    for c in range(NCORES):
        o = np.asarray(res.results[c]["o"], dtype=np.float32)
        # [NG, NT, 2, 128, GC] -> (g, vt, k, p, pin, l)
        a = o.reshape(NG, NT, 2, 128, GP, L)
        # pairs = g*GP + pin, b-major: b_local = pairs//C, ch = pairs%C
        a = a.transpose(0, 4, 2, 1, 3, 5).reshape(NPAIR, 2, N, L)
        a = a.reshape(BSH, C, 2, N, L).reshape(BSH, 2 * C, N, L)
        out[c * BSH:(c + 1) * BSH] = a
    if _trace:
        return out, res
    return out


# revision 12
# speedup vs baseline: 1.1816x; 1.1816x over previous
"""GraphSAGE (2-layer, DGL SAGEConv-mean) Trainium2 kernel — y-scheme.

Data-parallel over B (4 samples per core, 8 cores). Per (b,c) pair, with
A=adj, deg=max(indeg,1), D=diag(deg):

  y  = A^T x                      (level Y, 24 cols/pair)
  [R1'|R4|R5] = y @ [A00|B01|C01] (PE transpose of y + small matmuls,
                                   output lands node-major directly)
  t  = A^T (D^{-1} R5)            (level T2)
  w  = A^T R5                     (level W)
  OUT0 = dinv4*(t + R4) + (4*x@A00 + biasN)        [host-folded mb0]
  OUT1 = dinv*(A^T (R4 + D^{-1} w)) + R1' + biasN  (level A1)

vs the previous 6-level scheme this applies A^T to 4 slabs per pair
instead of 6 (96 vs 144 moving cols/pair). adj is stored fp8_e4m3
(exact for 0/1), halving its SBUF/DMA footprint. Small-weight products
use lhsT = y^T chunks so results come out node-major (no back-transpose).
"""
import sys

sys.path.insert(0, "/opt/trn_rl_repo")

import numpy as np
import ml_dtypes

from concourse import bass, bacc, tile, mybir
from concourse.bass_utils import run_bass_kernel_spmd

BF16 = mybir.dt.bfloat16
F32 = mybir.dt.float32
FP8 = mybir.dt.float8e4

N = 2048
L = 24
B = 32
C = 8
NCORES = 8
BSH = B // NCORES          # 4 samples per core
NPAIR = BSH * C            # 32 (b,c) pairs per core
NT = N // 128              # 16 node tiles
NG = 2                     # pair groups per core
GP = NPAIR // NG           # 16 pairs per group
GC = GP * L                # 384 moving columns per group
NSLAB = 4                  # transpose slabs per group (4 pairs each)
SP = GP // NSLAB           # pairs per slab
SW = SP * L                # 96 columns per slab

_CACHE = {}


def _build_bass():
    nc = bacc.Bacc(
        "TRN2", target_bir_lowering=False, debug=False, num_devices=NCORES)
    adjb = nc.declare_dram_parameter("adjb", [128, NT * N], BF16, isOutput=False)
    xsd = nc.declare_dram_parameter("xs", [NG, 128, NT * GC], BF16, isOutput=False)
    mbd = nc.declare_dram_parameter("mb0", [NG, 128, NT * GC], BF16, isOutput=False)
    dinvd = nc.declare_dram_parameter("dinv", [128, NT], F32, isOutput=False)
    dinv4d = nc.declare_dram_parameter("dinv4", [128, NT], F32, isOutput=False)
    biasd = nc.declare_dram_parameter("biasN", [128, NT * GC], BF16, isOutput=False)
    wpd = nc.declare_dram_parameter("wp", [128, 72], BF16, isOutput=False)
    idd = nc.declare_dram_parameter("ident", [128, 128], BF16, isOutput=False)
    od = nc.declare_dram_parameter("o", [NG, NT, 2, 128, GC], F32, isOutput=True)

    mult = mybir.AluOpType.mult
    add = mybir.AluOpType.add

    with tile.TileContext(nc) as tc:
        with (
            tc.tile_pool(name="cst", bufs=1) as cst,
            tc.tile_pool(name="adjp", bufs=1) as adjp,
            tc.tile_pool(name="mov", bufs=1) as mov,
            tc.tile_pool(name="ysp", bufs=1) as ysp,
            tc.tile_pool(name="ytp", bufs=1) as ytp,
            tc.tile_pool(name="rap", bufs=1) as rap,
            tc.tile_pool(name="wrk", bufs=1) as wrk,
            tc.tile_pool(name="otp", bufs=2) as otp,
            tc.tile_pool(name="psY", bufs=2, space="PSUM") as psY,
            tc.tile_pool(name="psT", bufs=2, space="PSUM") as psT,
            tc.tile_pool(name="psS", bufs=2, space="PSUM") as psS,
            tc.tile_pool(name="psB", bufs=2, space="PSUM") as psB,
        ):
            adj_sb = adjp.tile([128, NT * N], BF16)
            nc.sync.dma_start(adj_sb[:], adjb[:])
            dinv_sb = cst.tile([128, NT], F32, tag="dinv")
            nc.sync.dma_start(dinv_sb[:], dinvd[:])
            dinv4_sb = cst.tile([128, NT], F32, tag="dinv4")
            nc.sync.dma_start(dinv4_sb[:], dinv4d[:])
            bias_sb = cst.tile([128, NT * GC], BF16, tag="biasN")
            nc.sync.dma_start(bias_sb[:], biasd[:])
            wp_sb = cst.tile([128, 72], BF16, tag="wp")
            nc.sync.dma_start(wp_sb[:], wpd[:])
            id_sb = cst.tile([128, 128], BF16, tag="ident")
            nc.sync.dma_start(id_sb[:], idd[:])

            def astile(u, vt):
                col = u * N + vt * 128
                return adj_sb[:, col:col + 128]

            for g in range(NG):
                xg = mov.tile([128, NT * GC], BF16, tag="xg")
                mb = mov.tile([128, NT * GC], BF16, tag="mb")
                nc.sync.dma_start(xg[:], xsd[g])
                nc.sync.dma_start(mb[:], mbd[g])

                # Level Y: y = A^T x (node-major)
                ys = ysp.tile([128, NT * GC], BF16, tag="ys")
                for vt in range(NT):
                    ps = psY.tile([128, GC], F32)
                    for u in range(NT):
                        nc.tensor.matmul(
                            ps[:], astile(u, vt), xg[:, u * GC:(u + 1) * GC],
                            start=(u == 0), stop=(u == NT - 1))
                    nc.vector.tensor_copy(ys[:, vt * GC:(vt + 1) * GC], ps[:])

                # Transpose y -> y^T slabs (4 pairs per slab)
                yts = [
                    ytp.tile([128, NT * 128], BF16, tag=f"yt{s}",
                             name=f"yt{s}")
                    for s in range(NSLAB)
                ]
                # pair sp within a slab sits at partition base 32*sp (PE
                # tile positions must be 32-aligned)
                for ut in range(NT):
                    for s in range(NSLAB):
                        pt = psT.tile([128, 128], BF16)
                        for sp in range(SP):
                            p = s * SP + sp
                            nc.tensor.transpose(
                                pt[32 * sp:32 * sp + L, :],
                                ys[:, ut * GC + p * L: ut * GC + (p + 1) * L],
                                id_sb[:], tile_position=(0, 32 * sp))
                            nc.vector.tensor_copy(
                                yts[s][32 * sp:32 * sp + L,
                                       ut * 128:(ut + 1) * 128],
                                pt[32 * sp:32 * sp + L, :])

                # Smalls: [R1'|R4|R5](tile ut) = (y^T chunk)^T @ wp, node-major out
                rall = rap.tile([128, NT, GP, 72], BF16, tag="rall")
                for p in range(GP):
                    s, sp = divmod(p, SP)
                    for ut in range(NT):
                        pm = psS.tile([128, 72], F32)
                        nc.tensor.matmul(
                            pm[:],
                            yts[s][32 * sp:32 * sp + L,
                                   ut * 128:(ut + 1) * 128],
                            wp_sb[32 * sp:32 * sp + L, :],
                            tile_position=(32 * sp, 0))
                        nc.vector.tensor_copy(rall[:, ut, p, :], pm[:])

                # D5 = dinv * R5 (per source node)
                d5 = wrk.tile([128, NT * GC], BF16, tag="d5")
                for ut in range(NT):
                    nc.vector.tensor_scalar_mul(
                        d5[:, ut * GC:(ut + 1) * GC],
                        rall[:, ut, :, 48:72], dinv_sb[:, ut:ut + 1])

                # Level W: w = A^T R5; U2s = R4 + dinv*w
                u2s = wrk.tile([128, NT * GC], BF16, tag="u2s")
                for vt in range(NT):
                    ps = psB.tile([128, GC], F32)
                    for u in range(NT):
                        nc.tensor.matmul(
                            ps[:], astile(u, vt), rall[:, u, :, 48:72],
                            start=(u == 0), stop=(u == NT - 1))
                    nc.vector.scalar_tensor_tensor(
                        u2s[:, vt * GC:(vt + 1) * GC], ps[:],
                        dinv_sb[:, vt:vt + 1], rall[:, vt, :, 24:48],
                        op0=mult, op1=add)

                # Level T2: t = A^T D5; OUT0 = dinv4*(t + R4) + mb0
                for vt in range(NT):
                    ps = psB.tile([128, GC], F32)
                    for u in range(NT):
                        nc.tensor.matmul(
                            ps[:], astile(u, vt), d5[:, u * GC:(u + 1) * GC],
                            start=(u == 0), stop=(u == NT - 1))
                    t0 = otp.tile([128, GC], F32, tag="t0")
                    nc.vector.tensor_tensor(
                        t0[:], ps[:], rall[:, vt, :, 24:48], op=add)
                    t0b = otp.tile([128, GC], F32, tag="t0b")
                    nc.vector.scalar_tensor_tensor(
                        t0b[:], t0[:], dinv4_sb[:, vt:vt + 1],
                        mb[:, vt * GC:(vt + 1) * GC], op0=mult, op1=add)
                    nc.sync.dma_start(od[g, vt, 0], t0b[:])

                # Level A1: a1 = A^T U2s; OUT1 = dinv*a1 + R1' + biasN
                for vt in range(NT):
                    ps = psB.tile([128, GC], F32)
                    for u in range(NT):
                        nc.tensor.matmul(
                            ps[:], astile(u, vt), u2s[:, u * GC:(u + 1) * GC],
                            start=(u == 0), stop=(u == NT - 1))
                    t1 = otp.tile([128, GC], F32, tag="t1")
                    nc.vector.scalar_tensor_tensor(
                        t1[:], ps[:], dinv_sb[:, vt:vt + 1],
                        rall[:, vt, :, 0:24], op0=mult, op1=add)
                    t1b = otp.tile([128, GC], F32, tag="t1b")
                    nc.vector.tensor_tensor(
                        t1b[:], t1[:], bias_sb[:, vt * GC:(vt + 1) * GC], op=add)
                    nc.sync.dma_start(od[g, vt, 1], t1b[:])
    nc.compile()
    return nc


def _pack_moving(m):
    """[BSH, C, N, L] f32 -> [NG, 128, NT*GC] bf16 (pairs b-major)."""
    a = m.transpose(2, 0, 1, 3).reshape(NT, 128, NPAIR * L)
    a = a.reshape(NT, 128, NG, GC).transpose(2, 1, 0, 3).reshape(NG, 128, NT * GC)
    return np.ascontiguousarray(a).astype(ml_dtypes.bfloat16)


def kernel(x, adj, W_self, W_neigh, bias, _trace=False):
    x = np.asarray(x, dtype=np.float32)
    adj = np.asarray(adj, dtype=np.float32)
    W_self = np.asarray(W_self, dtype=np.float32)
    W_neigh = np.asarray(W_neigh, dtype=np.float32)
    bias = np.asarray(bias, dtype=np.float32)

    A00 = W_self[0].T @ W_self[1].T
    B01 = W_neigh[0].T @ W_self[1].T + W_self[0].T @ W_neigh[1].T
    C01 = W_neigh[0].T @ W_neigh[1].T
    indeg = adj.sum(0)
    deg = np.maximum(indeg, 1.0)
    s = (indeg >= 1).astype(np.float32)
    biasN = (bias[0] @ W_self[1].T + bias[1])[None, :] \
        + s[:, None] * (bias[0] @ W_neigh[1].T)[None, :]      # [N, L]

    adjb = np.ascontiguousarray(
        adj.reshape(NT, 128, N).transpose(1, 0, 2).reshape(128, NT * N)
    ).astype(ml_dtypes.bfloat16)
    dinv = np.ascontiguousarray((1.0 / deg).reshape(NT, 128).T).astype(np.float32)
    dinv4 = np.ascontiguousarray(4.0 * dinv)
    biasP = np.ascontiguousarray(
        np.broadcast_to(biasN.reshape(NT, 128, 1, L), (NT, 128, GP, L))
        .reshape(NT, 128, GC).transpose(1, 0, 2).reshape(128, NT * GC)
    ).astype(ml_dtypes.bfloat16)
    wp1 = np.concatenate([A00, B01, C01], axis=1)        # [24, 72]
    wp = np.zeros((128, 72), dtype=np.float32)
    for k in range(4):
        wp[32 * k:32 * k + L] = wp1
    wp = wp.astype(ml_dtypes.bfloat16)
    ident = np.eye(128, dtype=np.float32).astype(ml_dtypes.bfloat16)

    mb_all = 4.0 * (x @ A00) + biasN[None, None]

    if "nc" not in _CACHE:
        _CACHE["nc"] = _build_bass()
    nc = _CACHE["nc"]

    in_maps = []
    for c in range(NCORES):
        sl = slice(c * BSH, (c + 1) * BSH)
        in_maps.append({
            "adjb": adjb,
            "xs": _pack_moving(x[sl]),
            "mb0": _pack_moving(mb_all[sl]),
            "dinv": dinv,
            "dinv4": dinv4,
            "biasN": biasP,
            "wp": wp,
            "ident": ident,
        })

    res = run_bass_kernel_spmd(
        nc, in_maps, list(range(NCORES)), trace=_trace)

    out = np.empty((B, 2 * C, N, L), dtype=np.float32)
    for c in range(NCORES):
        o = np.asarray(res.results[c]["o"], dtype=np.float32)
        # [NG, NT, 2, 128, GC] -> (g, vt, k, p, pin, l)
        a = o.reshape(NG, NT, 2, 128, GP, L)
        # pairs = g*GP + pin, b-major: b_local = pairs//C, ch = pairs%C
        a = a.transpose(0, 4, 2, 1, 3, 5).reshape(NPAIR, 2, N, L)
        a = a.reshape(BSH, C, 2, N, L).reshape(BSH, 2 * C, N, L)
        out[c * BSH:(c + 1) * BSH] = a
    if _trace:
        return out, res
    return out
